# revision 7
# baseline (speedup 1.0000x reference)
"""Bahdanau attention + coverage GRU step, 8-core SPMD Trainium2 kernel.

Shapes (fixed): B=32, M=2048, QS=KS=H=1024, CS=128.
Sharding: data-parallel over batch, 4 batches per core, no collectives.

Algebra (reference math, reorganized):
  q   = query @ Wq.T + bq                                  [B, H]
  s   = tanh(q + mem @ Wk.T + cov @ Wc.T + (bk + bc)) @ Wl + bl
  e   = exp(s + maskbias);  Z = sum_m e;  w = e / Z
  gx  = cu @ Wih.T + bih
      = mem @ W2.T + cov @ W3.T + w*u + cb                 (fused weights)
        W2 = Wih @ Wkc, W3 = Wih @ Wcc, u = Wih @ Wac[:,0]
        cb = (query @ Wqc.T) @ Wih.T + [(bqc+bkc+bcc+bac) @ Wih.T + bih]
  gh  = cov @ Whh.T + bhh
  r,z = sigmoid(gx[:,:2C] + gh[:,:2C]);  n = tanh(gx[:,2C:] + r*gh[:,2C:])
  ncov = n + z*(cov - n)
  attns = w @ mem
"""

import numpy as np

B, M, QS, KS, H, CS = 32, 2048, 1024, 1024, 1024, 128
G = 3 * CS  # 384
N_CORES = 8
B_LOC = B // N_CORES  # 4
KC = 8  # k chunks of 128 (H = KS = QS = 1024)

_cache = {}


def _build_nc(b_loc, m_len):
    import concourse.bacc as bacc
    import concourse.mybir as mybir
    from concourse import tile
    from concourse.alu_op_type import AluOpType

    f32 = mybir.dt.float32
    f32r = mybir.dt.float32r
    AF = mybir.ActivationFunctionType
    MT = m_len // 128  # m tiles per batch

    nc = bacc.Bacc("TRN2", target_bir_lowering=False, debug=False,
                   num_devices=N_CORES)

    def din(name, shape):
        return nc.dram_tensor(name, shape, f32, kind="ExternalInput").ap()

    def dout(name, shape):
        return nc.dram_tensor(name, shape, f32, kind="ExternalOutput").ap()

    # -------- external tensors (per-core shapes) --------
    memT_d = din("memT", [b_loc, KS, m_len])      # memory[b].T
    mem_d = din("mem", [b_loc, m_len, KS])        # natural
    covT_d = din("covT", [b_loc, CS, m_len])      # coverage[b].T
    cov_d = din("cov", [b_loc, m_len, CS])        # natural
    qT_d = din("qT", [128, KC, b_loc])            # query.T chunked
    wk_d = din("wk", [128, KC, H])                # Wk.T chunked
    w2_d = din("w2", [128, KC, G])                # W2.T chunked
    wq_d = din("wq", [128, KC, H])                # Wq.T chunked
    wqc_d = din("wqc", [128, KC, H])              # Wqc.T chunked
    wih_d = din("wih", [128, KC, G])              # Wih.T chunked
    wc_d = din("wc", [CS, H])                     # Wc.T
    w3_d = din("w3", [CS, G])                     # W3.T
    whh_d = din("whh", [CS, G])                   # Whh.T
    wl_d = din("wl", [H])
    u_d = din("u", [G])
    qbias_d = din("qbias", [H])                   # bq + bk + bc
    cbias_d = din("cbias", [G])                   # bias part of cb
    bhh_d = din("bhh", [G])
    maskb_d = din("maskb", [b_loc, 128, MT])      # -1e18 / bl0
    eye4_d = din("eye4", [4, 4])

    attns_d = dout("attns", [b_loc, KS])
    wout_d = dout("wout", [b_loc, m_len])
    ncov_d = dout("ncov", [b_loc, m_len, CS])

    with tile.TileContext(nc) as tc:
        with (
            tc.tile_pool(name="wpool", bufs=1) as wp,
            tc.tile_pool(name="mTp", bufs=3) as mTp,
            tc.tile_pool(name="covTp", bufs=3) as covTp,
            tc.tile_pool(name="tp", bufs=3) as tp,
            tc.tile_pool(name="g2keep", bufs=MT + 2) as g2p,
            tc.tile_pool(name="batchp", bufs=2) as bp,
            tc.tile_pool(name="smallp", bufs=2) as smp,
            tc.tile_pool(name="pass2p", bufs=2) as p2p,
            tc.tile_pool(name="natp", bufs=2) as natp,
            tc.tile_pool(name="ps_score", bufs=3, space="PSUM") as ps_s,
            tc.tile_pool(name="ps_small", bufs=2, space="PSUM") as ps_sm,
            tc.tile_pool(name="ps_att", bufs=1, space="PSUM") as ps_a,
        ):
            # -------- resident weights --------
            wk_sb = wp.tile([128, KC, H], f32r)
            nc.sync.dma_start(out=wk_sb[:], in_=wk_d.bitcast(f32r))
            w2_sb = wp.tile([128, KC, G], f32r)
            nc.sync.dma_start(out=w2_sb[:], in_=w2_d.bitcast(f32r))
            wc_sb = wp.tile([CS, H], f32r)
            nc.sync.dma_start(out=wc_sb[:], in_=wc_d.bitcast(f32r))
            w3_sb = wp.tile([CS, G], f32r)
            nc.sync.dma_start(out=w3_sb[:], in_=w3_d.bitcast(f32r))
            whh_sb = wp.tile([CS, G], f32r)
            nc.sync.dma_start(out=whh_sb[:], in_=whh_d.bitcast(f32r))
            wl_rep = wp.tile([128, H], f32)
            nc.sync.dma_start(out=wl_rep[:], in_=wl_d[None, :].to_broadcast((128, H)))
            u_rep = wp.tile([128, G], f32)
            nc.sync.dma_start(out=u_rep[:], in_=u_d[None, :].to_broadcast((128, G)))
            bhh_row = wp.tile([1, G], f32r)
            nc.sync.dma_start(out=bhh_row[:], in_=bhh_d[None, :].bitcast(f32r))
            maskb_sb = wp.tile([128, b_loc, MT], f32)
            nc.sync.dma_start(out=maskb_sb[:], in_=maskb_d.rearrange("b p t -> p b t"))
            eye4 = wp.tile([4, 4], f32)
            nc.sync.dma_start(out=eye4[:], in_=eye4_d[:])
            qT_sb = wp.tile([128, KC, b_loc], f32r)
            nc.sync.dma_start(out=qT_sb[:], in_=qT_d.bitcast(f32r))
            ones_f32 = wp.tile([1, 128], f32)
            nc.vector.memset(ones_f32[:], 1.0)
            ones_row = wp.tile([1, 128], f32r)
            nc.vector.tensor_copy(ones_row[:], ones_f32[:])
            ones_col = wp.tile([128, 1], f32)
            nc.vector.memset(ones_col[:], 1.0)

            # -------- prologue: q = query @ WqT + qbias --------
            # stream weight chunks once, accumulate both 512-halves
            ps_q0 = ps_s.tile([b_loc, 512], f32, tag="score")
            ps_q1 = ps_s.tile([b_loc, 512], f32, tag="score")
            for c in range(KC):
                wq_t = mTp.tile([128, H], f32r, tag="mT")
                nc.sync.dma_start(out=wq_t[:], in_=wq_d[:, c, :].bitcast(f32r))
                lhs = qT_sb[:, c, :]
                nc.tensor.matmul(ps_q0[:], lhs, wq_t[:, 0:512],
                                 start=(c == 0), stop=(c == KC - 1))
                nc.tensor.matmul(ps_q1[:], lhs, wq_t[:, 512:1024],
                                 start=(c == 0), stop=(c == KC - 1))
            qb4 = smp.tile([b_loc, H], f32, tag="qsb")
            nc.sync.dma_start(out=qb4[:], in_=qbias_d[None, :].to_broadcast((b_loc, H)))
            q_sb = wp.tile([b_loc, H], f32)
            nc.vector.tensor_add(q_sb[:, 0:512], ps_q0[:], qb4[:, 0:512])
            nc.vector.tensor_add(q_sb[:, 512:1024], ps_q1[:], qb4[:, 512:1024])

            # qc_raw = query @ WqcT
            ps_c0 = ps_s.tile([b_loc, 512], f32, tag="score")
            ps_c1 = ps_s.tile([b_loc, 512], f32, tag="score")
            for c in range(KC):
                wqc_t = mTp.tile([128, H], f32r, tag="mT")
                nc.sync.dma_start(out=wqc_t[:], in_=wqc_d[:, c, :].bitcast(f32r))
                lhs = qT_sb[:, c, :]
                nc.tensor.matmul(ps_c0[:], lhs, wqc_t[:, 0:512],
                                 start=(c == 0), stop=(c == KC - 1))
                nc.tensor.matmul(ps_c1[:], lhs, wqc_t[:, 512:1024],
                                 start=(c == 0), stop=(c == KC - 1))
            qc_sb = wp.tile([b_loc, H], f32)
            nc.scalar.copy(qc_sb[:, 0:512], ps_c0[:])
            nc.scalar.copy(qc_sb[:, 512:1024], ps_c1[:])

            # transpose qc [b_loc, H] -> qcT [128, KC, b_loc]
            ps_tr = ps_sm.tile([128, KC * b_loc], f32, tag="small")
            for c in range(KC):
                nc.tensor.transpose(ps_tr[:, c * b_loc:(c + 1) * b_loc],
                                    qc_sb[0:b_loc, c * 128:(c + 1) * 128],
                                    eye4[0:b_loc, 0:b_loc])
            qcT_sb = wp.tile([128, KC * b_loc], f32r)
            nc.scalar.copy(qcT_sb[:], ps_tr[:])

            # cb = qcT.T @ WihT + cbias
            ps_cb = ps_sm.tile([b_loc, G], f32, tag="small")
            for c in range(KC):
                wih_t = mTp.tile([128, G], f32r, tag="mT")
                nc.sync.dma_start(out=wih_t[:], in_=wih_d[:, c, :].bitcast(f32r))
                nc.tensor.matmul(ps_cb[:],
                                 qcT_sb[:, c * b_loc:(c + 1) * b_loc],
                                 wih_t[:],
                                 start=(c == 0), stop=(c == KC - 1))
            cbias4 = smp.tile([b_loc, G], f32, tag="qsb")
            nc.sync.dma_start(out=cbias4[:], in_=cbias_d[None, :].to_broadcast((b_loc, G)))
            cb_sb = wp.tile([b_loc, G], f32)
            nc.vector.tensor_add(cb_sb[:], ps_cb[:], cbias4[:, 0:G])

            # -------- per-batch main loops --------
            for b in range(b_loc):
                q_row = bp.tile([1, H], f32r, tag="qrow")
                nc.sync.dma_start(out=q_row[:], in_=q_sb[b:b + 1, :].bitcast(f32r))
                cb_row = bp.tile([1, G], f32r, tag="cbrow")
                nc.sync.dma_start(out=cb_row[:], in_=cb_sb[b:b + 1, :].bitcast(f32r))

                scores_b = bp.tile([128, MT], f32, tag="scores")
                e_b = bp.tile([128, MT], f32, tag="e")
                g2_tiles = []

                # ---- pass 1 ----
                for mt in range(MT):
                    mT_t = mTp.tile([128, KC, 128], f32r, tag="mT")
                    nc.sync.dma_start(
                        out=mT_t[:],
                        in_=memT_d[b].bitcast(f32r).rearrange("(c p) m -> p c m", p=128)[
                            :, :, mt * 128:(mt + 1) * 128])
                    covT_t = covTp.tile([CS, 128], f32r)
                    nc.sync.dma_start(out=covT_t[:],
                                      in_=covT_d[b].bitcast(f32r)[:, mt * 128:(mt + 1) * 128])

                    # scores psum halves
                    t_full = tp.tile([128, 1024], f32, tag="t")
                    for nh in range(2):
                        sl = slice(nh * 512, (nh + 1) * 512)
                        ps_h = ps_s.tile([128, 512], f32, tag="score")
                        nc.tensor.matmul(ps_h[:], ones_row[:],
                                         q_row[:, sl],
                                         start=True, stop=False)
                        nc.tensor.matmul(ps_h[:], covT_t[:],
                                         wc_sb[:, sl],
                                         start=False, stop=False)
                        for c in range(KC):
                            nc.tensor.matmul(
                                ps_h[:], mT_t[:, c, :],
                                wk_sb[:, c, sl],
                                start=False, stop=(c == KC - 1))
                        nc.scalar.activation(t_full[:, sl], ps_h[:], AF.Tanh)

                    # g2 psum
                    ps_g2 = ps_sm.tile([128, G], f32, tag="small")
                    nc.tensor.matmul(ps_g2[:], ones_row[:],
                                     cb_row[:], start=True, stop=False)
                    nc.tensor.matmul(ps_g2[:], covT_t[:],
                                     w3_sb[:], start=False, stop=False)
                    for c in range(KC):
                        nc.tensor.matmul(ps_g2[:], mT_t[:, c, :],
                                         w2_sb[:, c, :],
                                         start=False, stop=(c == KC - 1))
                    g2_t = g2p.tile([128, G], f32, tag="g2")
                    nc.scalar.copy(g2_t[:], ps_g2[:])
                    g2_tiles.append(g2_t)

                    # scores = sum(t * wl)
                    nc.vector.tensor_mul(t_full[:], t_full[:], wl_rep[:])
                    nc.vector.reduce_sum(scores_b[:, mt:mt + 1], t_full[:],
                                         axis=mybir.AxisListType.X)
                    # e = exp(score + maskbias)
                    nc.scalar.activation(e_b[:, mt:mt + 1], scores_b[:, mt:mt + 1],
                                         AF.Exp, bias=maskb_sb[:, b, mt:mt + 1])

                # ---- softmax normalizer ----
                zp = bp.tile([128, 1], f32, tag="zp")
                nc.vector.reduce_sum(zp[:], e_b[:], axis=mybir.AxisListType.X)
                ps_z = ps_a.tile([1, 1], f32, tag="z")
                nc.tensor.matmul(ps_z[:], zp[:], ones_col[:], start=True, stop=True)
                zinv = bp.tile([1, 1], f32, tag="zinv")
                nc.vector.reciprocal(zinv[:], ps_z[:])
                zrep = bp.tile([128, 1], f32, tag="zrep")
                nc.gpsimd.partition_broadcast(zrep[:], zinv[:])
                w_sb = bp.tile([128, MT], f32, tag="w")
                nc.vector.tensor_mul(w_sb[:], e_b[:], zrep[:].to_broadcast((128, MT)))
                w_r = bp.tile([128, MT], f32r, tag="wr")
                nc.vector.tensor_copy(w_r[:], w_sb[:])
                nc.sync.dma_start(out=wout_d[b].rearrange("(t p) -> p t", p=128),
                                  in_=w_sb[:])

                attn_acc = bp.tile([1, KS], f32, tag="attacc")
                nc.vector.memset(attn_acc[:], 0.0)

                # ---- pass 2 ----
                for mt in range(MT):
                    sl_m = slice(mt * 128, (mt + 1) * 128)
                    covT_t = covTp.tile([CS, 128], f32r)
                    nc.sync.dma_start(out=covT_t[:], in_=covT_d[b].bitcast(f32r)[:, sl_m])
                    cov_nat = natp.tile([128, CS], f32, tag="covnat")
                    nc.sync.dma_start(out=cov_nat[:], in_=cov_d[b][sl_m, :])
                    mem_nat = natp.tile([128, KS], f32r, tag="memnat")
                    nc.sync.dma_start(out=mem_nat[:], in_=mem_d[b].bitcast(f32r)[sl_m, :])

                    # gh = cov @ WhhT + bhh
                    ps_gh = ps_sm.tile([128, G], f32, tag="small")
                    nc.tensor.matmul(ps_gh[:], ones_row[:],
                                     bhh_row[:], start=True, stop=False)
                    nc.tensor.matmul(ps_gh[:], covT_t[:],
                                     whh_sb[:], start=False, stop=True)

                    # gx = g2 + w*u
                    gx = p2p.tile([128, G], f32, tag="gx")
                    nc.vector.scalar_tensor_tensor(
                        out=gx[:], in0=u_rep[:], scalar=w_sb[:, mt:mt + 1],
                        in1=g2_tiles[mt][:],
                        op0=AluOpType.mult, op1=AluOpType.add)

                    # gates
                    rz_in = p2p.tile([128, 2 * CS], f32, tag="rzin")
                    nc.vector.tensor_add(rz_in[:], gx[:, 0:2 * CS], ps_gh[:, 0:2 * CS])
                    rz = p2p.tile([128, 2 * CS], f32, tag="rz")
                    nc.scalar.activation(rz[:], rz_in[:], AF.Sigmoid)
                    rhn = p2p.tile([128, CS], f32, tag="rhn")
                    nc.vector.tensor_mul(rhn[:], rz[:, 0:CS], ps_gh[:, 2 * CS:G])
                    n_in = p2p.tile([128, CS], f32, tag="nin")
                    nc.vector.tensor_add(n_in[:], gx[:, 2 * CS:G], rhn[:])
                    n_sb = p2p.tile([128, CS], f32, tag="nsb")
                    nc.scalar.activation(n_sb[:], n_in[:], AF.Tanh)
                    d_t = p2p.tile([128, CS], f32, tag="dt")
                    nc.vector.tensor_sub(d_t[:], cov_nat[:], n_sb[:])
                    zd = p2p.tile([128, CS], f32, tag="zd")
                    nc.vector.tensor_mul(zd[:], rz[:, CS:2 * CS], d_t[:])
                    ncov_t = p2p.tile([128, CS], f32, tag="ncovt")
                    nc.vector.tensor_add(ncov_t[:], n_sb[:], zd[:])
                    nc.sync.dma_start(out=ncov_d[b][sl_m, :], in_=ncov_t[:])

                    # attns partial: w_col.T @ mem_nat
                    for nh in range(2):
                        sl = slice(nh * 512, (nh + 1) * 512)
                        ps_at = ps_a.tile([1, 512], f32, tag="att")
                        nc.tensor.matmul(ps_at[:], w_r[:, mt:mt + 1],
                                         mem_nat[:, sl],
                                         start=True, stop=True)
                        nc.vector.tensor_add(attn_acc[:, sl], attn_acc[:, sl],
                                             ps_at[:])

                nc.sync.dma_start(out=attns_d[b][None, :], in_=attn_acc[:])

    nc.compile()
    return nc


def _host_prep(query, memory, coverage, Wq, bq, Wk, bk, Wc, bc, Wcc, bcc,
               Wqc, bqc, Wkc, bkc, Wac, bac, Wl, bl,
               gru_wih, gru_bih, gru_whh, gru_bhh, mask, n_cores=N_CORES):
    f32 = np.float32
    b_loc = memory.shape[0] // n_cores
    m_len = memory.shape[1]
    MT = m_len // 128

    def chunked(WT):  # [K, N] -> [128, KC, N]
        k = WT.shape[0]
        return np.ascontiguousarray(
            WT.reshape(k // 128, 128, WT.shape[1]).transpose(1, 0, 2))

    w64 = lambda a: a.astype(np.float64)
    W2 = (w64(gru_wih) @ w64(Wkc)).astype(f32)        # [G, KS]
    W3 = (w64(gru_wih) @ w64(Wcc)).astype(f32)        # [G, CS]
    u = (w64(gru_wih) @ w64(Wac[:, 0])).astype(f32)   # [G]
    cbias = ((w64(bqc + bkc + bcc + bac) @ w64(gru_wih).T) + w64(gru_bih)).astype(f32)
    qbias = (bq + bk + bc).astype(f32)

    wk_c = chunked(np.ascontiguousarray(Wk.T))
    w2_c = chunked(np.ascontiguousarray(W2.T))
    wq_c = chunked(np.ascontiguousarray(Wq.T))
    wqc_c = chunked(np.ascontiguousarray(Wqc.T))
    wih_c = chunked(np.ascontiguousarray(gru_wih.T))
    wc_t = np.ascontiguousarray(Wc.T)
    w3_t = np.ascontiguousarray(W3.T)
    whh_t = np.ascontiguousarray(gru_whh.T)
    eye4 = np.eye(4, dtype=f32)

    maskbias = np.where(mask, f32(-1e18), f32(bl[0]))  # [B, M]
    # [b, p, t] with m = t*128 + p
    maskb = np.ascontiguousarray(
        maskbias.reshape(-1, MT, 128).transpose(0, 2, 1)).astype(f32)

    memT = np.ascontiguousarray(memory.transpose(0, 2, 1))
    covT = np.ascontiguousarray(coverage.transpose(0, 2, 1))
    mem = np.ascontiguousarray(memory)
    cov = np.ascontiguousarray(coverage)

    in_maps = []
    for i in range(n_cores):
        sl = slice(i * b_loc, (i + 1) * b_loc)
        qT = np.ascontiguousarray(query[sl].T)  # [QS, b_loc]
        qT_c = np.ascontiguousarray(
            qT.reshape(KC, 128, b_loc).transpose(1, 0, 2))
        in_maps.append(dict(
            memT=memT[sl], mem=mem[sl], covT=covT[sl], cov=cov[sl],
            qT=qT_c, wk=wk_c, w2=w2_c, wq=wq_c, wqc=wqc_c, wih=wih_c,
            wc=wc_t, w3=w3_t, whh=whh_t, wl=np.ascontiguousarray(Wl[0]),
            u=u, qbias=qbias, cbias=cbias, bhh=gru_bhh.astype(f32),
            maskb=maskb[sl], eye4=eye4,
        ))
    return in_maps


def _get_nc(b_loc, m_len):
    key = (b_loc, m_len)
    if key not in _cache:
        _cache[key] = _build_nc(b_loc, m_len)
    return _cache[key]


def kernel(**inputs):
    from concourse.bass_utils import run_bass_kernel_spmd

    in_maps = _host_prep(**inputs)
    nc = _get_nc(B_LOC, M)
    res = run_bass_kernel_spmd(nc, in_maps, list(range(N_CORES)))
    attns = np.concatenate([r["attns"] for r in res.results], axis=0)
    weights = np.concatenate([r["wout"] for r in res.results], axis=0)
    ncov = np.concatenate([r["ncov"] for r in res.results], axis=0)
    return attns, weights, ncov
